# revision 5
# baseline (speedup 1.0000x reference)
"""Trainium2 Bass kernel: sliding-window causal MHA with RoPE + ALiBi.

Sharding: 8 cores = 4 batches x 2 head-groups (8 heads each).
All large matmuls run as fp8e4m3 DoubleRow (K=256/instruction) with 3-term
residual compensation (hi@hi + lo@hi + hi@lo), which matches bf16 accuracy at
half the PE row count; attention itself (scores/PV/sums) stays bf16.

Per-core program:
  A1: q/k projections, weights pre-scaled x64 (fp8 range), 1/4096 folded into
      the exp scale. Terms ordered x8-first so PE paces the x8/xlo DMA streams.
      RoPE on bf16 in half-split d-layout (4 DVE ops per head tile).
  A2: v projection (weights x32 so 32*attn stays under fp8 max 240), with an
      extra ones x bias contraction chunk folding bv into v. Interleaved with
      group 0 of attention to keep PE fed.
  C:  per (group, head): paired j-tiles -> one [128,512] scores psum -> exp
      (bf16) -> x expbP (separable ALiBi+window mask blocks, bf16) -> PV +
      ones-matmul sums into one shared psum bank -> reciprocal/mul normalize,
      emitted as fp8 hi (Act) + fp8 residual (DVE) pairs.
  D:  out projection in fp8 DoubleRow (3-term, wo pre-scaled x64), lagged one
      query group behind attention and spread across its head loop; host sums
      the 2 head-group partials + bo.
"""
import sys
sys.path.insert(0, '/opt/trn_rl_repo')

import numpy as np
import ml_dtypes
import concourse.bass as bass
import concourse.bacc as bacc
import concourse.mybir as mybir
import concourse.tile as tile

L, N, C, H, D, W = 1024, 4, 2048, 16, 128, 512
HPC = 8                       # heads per core
GD = HPC * D                  # 1024 head-dims per core
NKP = C // 256                # 8 contraction k-pairs (256 each)
SCALE = 1.0 / float(np.sqrt(D))
WS = 64.0                     # q/k/wo weight pre-scale for fp8 range
WS_V = 32.0                   # v-path pre-scale (keeps 32*attn under fp8e4m3 max 240)
F32 = mybir.dt.float32
BF16 = mybir.dt.bfloat16
FP8 = mybir.dt.float8e4
DRM = mybir.MatmulPerfMode.DoubleRow
AF = mybir.ActivationFunctionType
ALU = mybir.AluOpType
QG = 256
NQG = L // QG                 # 4
MASK_W = 1408
MASK_C0 = 384


def jtiles(i0):
    return list(range(max(0, i0 - W), min(i0 + QG, L) - 128 + 1, 128))


def pair2(i):
    return i.rearrange("p (two n) -> p two n", two=2)


def emit(tc, t):
    nc = tc.nc
    cpool = tc.alloc_tile_pool(name="const", bufs=1, side="left")
    bq_s = cpool.tile([128, HPC], F32, tag="bq")
    bk_s = cpool.tile([128, HPC], F32, tag="bk")
    ones = cpool.tile([128, 128], BF16, tag="ones")
    expbs = [cpool.tile([128, 1536], BF16, tag=f"eb{h}", name=f"eb{h}")
             for h in range(HPC)]

    # persistent left-stack results
    qkp = tc.alloc_tile_pool(name="qkp", bufs=1, side="left")
    qts = [qkp.tile([128, L], BF16, tag=f"q{m}", name=f"q{m}") for m in range(HPC)]
    kts = [qkp.tile([128, L], BF16, tag=f"k{m}", name=f"k{m}") for m in range(HPC)]
    vp = tc.alloc_tile_pool(name="vp", bufs=1, side="left")
    vts = [vp.tile([128, GD], BF16, tag=f"v{tt}", name=f"v{tt}") for tt in range(HPC)]
    # attention outputs as fp8 hi/lo pairs, head-pair-major for DR out-proj:
    # at8p[hp][p, (h%2)*1024 + t]
    atp = tc.alloc_tile_pool(name="atp", bufs=1, side="left")
    at8p = [atp.tile([128, 2048], FP8, tag=f"a8{hp}", name=f"a8{hp}")
            for hp in range(HPC // 2)]
    atlop = [atp.tile([128, 2048], FP8, tag=f"alo{hp}", name=f"alo{hp}")
             for hp in range(HPC // 2)]

    # fp8 inputs (right stack, released after v-proj); chunk NKP of x8/wv8 is
    # the ones/bias pair that folds bv into the v projection
    xp = tc.alloc_tile_pool(name="xp", bufs=1, side="right")
    x8s = xp.tile([128, (NKP + 1) * 2048], FP8, tag="x8")
    xlos = xp.tile([128, NKP * 2048], FP8, tag="xlo")
    wvp = tc.alloc_tile_pool(name="wvp", bufs=1, side="right")
    wv8s = wvp.tile([128, (NKP + 1) * 2048], FP8, tag="wv8")
    wvlos = wvp.tile([128, NKP * 2048], FP8, tag="wvlo")

    # ---------------- phase A1: q/k projections + rope ----------------
    # DMA order: x8 chunks, job0 weights, xlo chunks, constants, then
    # weights prefetched one (m, proj) job ahead. Within a job the chains
    # run term-major so the two x8-only terms overlap the xlo stream.
    with tc.tile_pool(name="csp", bufs=1, side="right") as csp, \
         tc.tile_pool(name="ws", bufs=6, side="right") as ws, \
         tc.tile_pool(name="rp", bufs=3, side="right") as rp, \
         tc.tile_pool(name="pa", bufs=6, space="PSUM") as pa:
        cos2 = csp.tile([128, L], BF16, tag="cos2")
        sinsw = csp.tile([128, L], BF16, tag="sinsw")
        def fetch_w(pair_m, interleave_x0=False):
            out = []
            for wname, wloname in (("wq8", "wqlo"), ("wk8", "wklo")):
                w8t = ws.tile([128, 2048], FP8, tag="w8", name="w8")
                nc.sync.dma_start(w8t[:], t[wname][pair_m])
                if interleave_x0 and wname == "wq8":
                    nc.sync.dma_start(x8s[:, 0:2048], t["x8"][0])
                wlot = ws.tile([128, 2048], FP8, tag="wlo", name="wlo")
                nc.sync.dma_start(wlot[:], t[wloname][pair_m])
                out.append((w8t, wlot))
            return out

        wq_fifo = [fetch_w(0, interleave_x0=True)]
        for kp in range(1, NKP + 1):
            nc.sync.dma_start(x8s[:, kp * 2048:(kp + 1) * 2048], t["x8"][kp])
        for kp in range(NKP):
            nc.sync.dma_start(xlos[:, kp * 2048:(kp + 1) * 2048], t["xlo"][kp])
        wq_fifo.append(fetch_w(1))
        nc.sync.dma_start(cos2[:], t["cos2"][:])
        nc.sync.dma_start(sinsw[:], t["sinsw"][:])
        nc.sync.dma_start(bq_s[:], t["bq"][:])
        nc.sync.dma_start(bk_s[:], t["bk"][:])
        nc.sync.dma_start(ones[:], t["ones"][:])

        for m in range(HPC):
            wpair = wq_fifo.pop(0)
            if m + 2 < HPC:
                wq_fifo.append(fetch_w(m + 2))
            # q and k jobs of this m: 4 open psums; the two x8-only terms of
            # both jobs run first so PE keeps pace with the x8/xlo streams.
            pss = [[pa.tile([128, 512], F32, tag="pp", name=f"ps{j}{hf}")
                    for hf in range(2)] for j in range(2)]
            nmm = [[0, 0], [0, 0]]

            def term(j, lhs, rhs, lastterm=False):
                for kp in range(NKP):
                    rh = pair2(rhs[:, kp * 2048:(kp + 1) * 2048])
                    lh = pair2(lhs[:, kp * 256:(kp + 1) * 256])
                    for hf in range(2):
                        for c2 in range(2):
                            tq = hf * 512 + c2 * 256
                            nc.tensor.matmul(
                                pss[j][hf][:, c2 * 256:(c2 + 1) * 256],
                                lh, rh[:, :, tq:tq + 256],
                                start=(nmm[j][hf] == 0),
                                stop=(lastterm and kp == NKP - 1 and c2 == 1),
                                perf_mode=DRM, skip_group_check=True)
                            nmm[j][hf] += 1

            term(0, wpair[0][0], x8s)         # q: w8 @ x8
            term(1, wpair[1][0], x8s)         # k: w8 @ x8
            term(0, wpair[0][1], x8s)         # q: wlo @ x8
            term(1, wpair[1][1], x8s)         # k: wlo @ x8
            term(0, wpair[0][0], xlos, True)  # q: w8 @ xlo
            term(1, wpair[1][0], xlos, True)  # k: w8 @ xlo

            for j, (bias_s, dst) in enumerate(((bq_s, qts), (bk_s, kts))):
                qw = rp.tile([128, L], BF16, tag="qw", name="qw")
                for hf in range(2):
                    nc.scalar.activation(
                        qw[:, hf * 512:(hf + 1) * 512], pss[j][hf][:],
                        AF.Identity, bias=bias_s[:, m:m + 1], scale=1.0)
                # rope on 64-scaled bf16: dst = qw*cos2 + swap(qw)*sin
                t1 = rp.tile([128, L], BF16, tag="t1", name="t1")
                nc.vector.tensor_mul(t1[:], qw[:], cos2[:])
                t2 = rp.tile([128, L], BF16, tag="t2", name="t2")
                nc.vector.tensor_mul(t2[0:64, :], qw[64:128, :], sinsw[64:128, :])
                nc.vector.tensor_mul(t2[64:128, :], qw[0:64, :], sinsw[0:64, :])
                nc.vector.tensor_add(dst[m][:], t1[:], t2[:])

        # wv weights stream in during A1 tail; expb masks for phase C
        for kp in range(NKP + 1):
            nc.sync.dma_start(wv8s[:, kp * 2048:(kp + 1) * 2048], t["wv8"][kp])
        for kp in range(NKP):
            nc.sync.dma_start(wvlos[:, kp * 2048:(kp + 1) * 2048], t["wvlo"][kp])
        for h in range(HPC):
            nc.sync.dma_start(expbs[h][:], t["expbP"][h])

    # out-proj weights on the left stack; DMAs issue right after A1
    wop = tc.alloc_tile_pool(name="wop", bufs=1, side="left")
    wo8t, wolot = [], []
    for hp in range(HPC // 2):
        w8 = wop.tile([128, 4096], FP8, tag="wo8{}".format(hp), name="wo8{}".format(hp))
        nc.sync.dma_start(w8[:], t["wo8"][hp])
        wo8t.append(w8)
    for hp in range(HPC // 2):
        wl = wop.tile([128, 4096], FP8, tag="wolo{}".format(hp), name="wolo{}".format(hp))
        nc.sync.dma_start(wl[:], t["wolo"][hp])
        wolot.append(wl)

    # ------- phases A2 + C + D: v-proj interleaved with group 0, then
    # attention with the out projection lagged one group behind -------
    with tc.tile_pool(name="cwA", bufs=4, side="right") as cwA, \
         tc.tile_pool(name="cwB", bufs=2, side="right") as cwB, \
         tc.tile_pool(name="og", bufs=3, side="right") as og, \
         tc.tile_pool(name="pcs", bufs=4, space="PSUM") as pcs, \
         tc.tile_pool(name="pca", bufs=2, space="PSUM") as pca:

        def attn_head(gi, h):
            i0 = gi * QG
            js = jtiles(i0)
            prs = [(js[2 * u], js[2 * u + 1]) for u in range(len(js) // 2)]
            # one bank: cols [0:256] = PV accum, cols [256:512] = sums accum
            asum = pca.tile([128, 512], F32, tag="asum", name="asum_ps")
            for u, (jlo, jhi) in enumerate(prs):
                bi = (MASK_C0 - (jhi - i0)) // 256 - 1   # 256,512,768 -> 0,1,2
                sp = pcs.tile([128, 512], F32, tag="s", name="s_ps")
                nc.tensor.matmul(sp[:, 0:256], kts[h][:, jhi:jhi + 128],
                                 qts[h][:, i0:i0 + QG], start=True, stop=False,
                                 skip_group_check=True)
                nc.tensor.matmul(sp[:, 256:512], kts[h][:, jlo:jlo + 128],
                                 qts[h][:, i0:i0 + QG], start=False, stop=True,
                                 skip_group_check=True)
                e = cwA.tile([128, 512], BF16, tag="e", name="e")
                nc.scalar.activation(e[:], sp[:], AF.Exp, scale=SCALE / (WS * WS))
                pT = cwA.tile([128, 512], BF16, tag="pT", name="pT")
                nc.vector.tensor_mul(pT[:], e[:], expbs[h][:, bi * 512:(bi + 1) * 512])
                last = (u == len(prs) - 1)
                nc.tensor.matmul(asum[:, 0:256], vts[jhi // 128][:, h * 128:(h + 1) * 128],
                                 pT[:, 0:256], start=(u == 0), stop=False,
                                 skip_group_check=True)
                nc.tensor.matmul(asum[:, 0:256], vts[jlo // 128][:, h * 128:(h + 1) * 128],
                                 pT[:, 256:512], start=False, stop=False,
                                 skip_group_check=True)
                nc.tensor.matmul(asum[:, 256:512], ones[:], pT[:, 0:256],
                                 start=False, stop=False, skip_group_check=True)
                nc.tensor.matmul(asum[:, 256:512], ones[:], pT[:, 256:512],
                                 start=False, stop=last, skip_group_check=True)
            rec = cwB.tile([128, QG], F32, tag="rec", name="rec")
            nc.vector.reciprocal(rec[:], asum[:, 256:512])
            awf = cwB.tile([128, QG], F32, tag="awf", name="awf")
            nc.vector.tensor_mul(awf[:], asum[:, 0:256], rec[:])
            a8sl = at8p[h // 2][:, (h % 2) * 1024 + i0:(h % 2) * 1024 + i0 + QG]
            nc.scalar.activation(a8sl, awf[:], AF.Identity, scale=1.0)
            nc.vector.tensor_sub(
                atlop[h // 2][:, (h % 2) * 1024 + i0:(h % 2) * 1024 + i0 + QG],
                awf[:], a8sl)

        with tc.tile_pool(name="pv", bufs=2, space="PSUM") as pv:
            def v_half(tt, hf):
                ps = pv.tile([128, 512], F32, tag="pp", name="psV")
                nmm2 = 0
                nlast = 2 * (3 * NKP + 1) - 1
                for c2 in range(2):
                    hd0 = hf * 512 + c2 * 256
                    for lhs, rhs, nk in ((x8s, wv8s, NKP + 1), (xlos, wv8s, NKP),
                                         (x8s, wvlos, NKP)):
                        for kp in range(nk):
                            nc.tensor.matmul(
                                ps[:, c2 * 256:(c2 + 1) * 256],
                                pair2(lhs[:, kp * 2048:(kp + 1) * 2048])[:, :, tt * 128:(tt + 1) * 128],
                                pair2(rhs[:, kp * 2048:(kp + 1) * 2048])[:, :, hd0:hd0 + 256],
                                start=(nmm2 == 0), stop=(nmm2 == nlast),
                                perf_mode=DRM, skip_group_check=True)
                            nmm2 += 1
                nc.vector.tensor_copy(vts[tt][:, hf * 512:(hf + 1) * 512], ps[:])

            for tt in (0, 1):
                for hf in range(2):
                    v_half(tt, hf)
            # remaining 12 v half-tiles spread over all 8 gi0 head steps
            sched = [2, 2, 2, 2, 1, 1, 1, 1]
            nxt = 4   # half-tile index (tt = nxt // 2, hf = nxt % 2)
            for h in range(HPC):
                for _ in range(sched[h]):
                    v_half(nxt // 2, nxt % 2)
                    nxt += 1
                attn_head(0, h)

        with tc.tile_pool(name="pd", bufs=2, space="PSUM") as pd:
            def emit_d(gi, idx):
                tt = 2 * gi + idx // 4
                cc = idx % 4
                # fp8 DR out-proj: psum [128,512] = two 256-col chains;
                # each chain: 3 terms x 4 head-pairs, K=256/instr
                ps = pd.tile([128, 512], F32, tag="po", name="psD")
                nmm3 = 0
                for c2 in range(2):
                    c0 = cc * 512 + c2 * 256
                    for lhsl, rhsl in ((at8p, wo8t), (at8p, wolot), (atlop, wo8t)):
                        for hp in range(HPC // 2):
                            nc.tensor.matmul(
                                ps[:, c2 * 256:(c2 + 1) * 256],
                                pair2(lhsl[hp][:, :])[:, :, tt * 128:(tt + 1) * 128],
                                pair2(rhsl[hp][:, :])[:, :, c0:c0 + 256],
                                start=(nmm3 == 0), stop=(nmm3 == 23),
                                perf_mode=DRM, skip_group_check=True)
                            nmm3 += 1
                split = (gi == NQG - 1 and idx == HPC - 1)
                o = og.tile([128, 512], F32, tag="o", name="o")
                if split:
                    # separate evac+DMA per 256-half to shorten the tail
                    for c2 in range(2):
                        sl = slice(c2 * 256, (c2 + 1) * 256)
                        nc.scalar.activation(o[:, sl], ps[:, sl], AF.Identity,
                                             scale=1.0 / (WS_V * WS))
                        nc.sync.dma_start(
                            t["out"][tt * 128:(tt + 1) * 128,
                                     cc * 512 + c2 * 256:cc * 512 + (c2 + 1) * 256],
                            o[:, sl])
                elif gi == NQG - 1:
                    # Act is idle after the last exp; keep DVE free to drain
                    nc.scalar.activation(o[:], ps[:], AF.Identity,
                                         scale=1.0 / (WS_V * WS))
                    nc.sync.dma_start(
                        t["out"][tt * 128:(tt + 1) * 128, cc * 512:(cc + 1) * 512], o[:])
                else:
                    nc.vector.tensor_scalar_mul(o[:], ps[:], 1.0 / (WS_V * WS))
                    nc.sync.dma_start(
                        t["out"][tt * 128:(tt + 1) * 128, cc * 512:(cc + 1) * 512], o[:])

            for gi in range(1, NQG):
                for h in range(HPC):
                    attn_head(gi, h)
                    emit_d(gi - 1, h)
            for idx in range(HPC):
                emit_d(NQG - 1, idx)

    wvp.release()
    xp.release()
    wop.release()
    atp.release()
    vp.release()
    qkp.release()
    cpool.release()


def build_nc(reps=1):
    nc = bacc.Bacc("TRN2", target_bir_lowering=False, debug=False,
                   enable_asserts=False, num_devices=8)
    t = {}
    for name in ("x8", "xlo", "wq8", "wqlo", "wk8", "wklo", "wv8", "wvlo"):
        n0 = 9 if name in ("x8", "wv8") else 8
        t[name] = nc.dram_tensor(name, [n0, 128, 2048], FP8, kind="ExternalInput").ap()
    t["wo8"] = nc.dram_tensor("wo8", [HPC // 2, 128, 4096], FP8, kind="ExternalInput").ap()
    t["wolo"] = nc.dram_tensor("wolo", [HPC // 2, 128, 4096], FP8, kind="ExternalInput").ap()
    t["cos2"] = nc.dram_tensor("cos2", [128, L], BF16, kind="ExternalInput").ap()
    t["sinsw"] = nc.dram_tensor("sinsw", [128, L], BF16, kind="ExternalInput").ap()
    t["bq"] = nc.dram_tensor("bq", [128, HPC], F32, kind="ExternalInput").ap()
    t["bk"] = nc.dram_tensor("bk", [128, HPC], F32, kind="ExternalInput").ap()
    t["expbP"] = nc.dram_tensor("expbP", [HPC, 128, 1536], BF16, kind="ExternalInput").ap()
    t["ones"] = nc.dram_tensor("ones", [128, 128], BF16, kind="ExternalInput").ap()
    t["out"] = nc.dram_tensor("out", [L, C], F32, kind="ExternalOutput").ap()
    with tile.TileContext(nc) as tc:
        for _ in range(reps):
            emit(tc, t)
    nc.compile()
    return nc


FP8NP = ml_dtypes.float8_e4m3
BF16NP = ml_dtypes.bfloat16


def q8(a):
    return a.astype(FP8NP)


def pack_x(xb):
    """xb: (L, C) f32 -> hi/lo fp8 [8, 128, 2048]; [kp][p, j*1024+t]."""
    xT = np.ascontiguousarray(xb.T)            # (C, L)
    x8 = q8(xT)
    xlo = q8(xT - x8.astype(np.float32))
    def pk(a):
        return np.ascontiguousarray(
            a.reshape(NKP, 2, 128, L).transpose(0, 2, 1, 3)).reshape(NKP, 128, 2 * L)
    return pk(x8), pk(xlo)


def pack_wqk(wg):
    """wg: (GD, C) f32 (x64-scaled rows) -> hi/lo fp8 [8 m, 128, 2048];
    [m][p, kp*256 + j*128 + h]."""
    w8 = q8(wg)
    wlo = q8(wg - w8.astype(np.float32))
    def pk(a):
        arr = a.reshape(HPC, 128, NKP, 2, 128)           # m, h, kp, j, p
        return np.ascontiguousarray(arr.transpose(0, 4, 2, 3, 1)).reshape(HPC, 128, 2048)
    return pk(w8), pk(wlo)


def pack_wv(wg):
    """wg: (GD, C) f32 (x64-scaled) -> hi/lo fp8 [8 kp, 128, 2048];
    [kp][p, j*1024 + hd]."""
    w8 = q8(wg)
    wlo = q8(wg - w8.astype(np.float32))
    def pk(a):
        arr = a.reshape(GD, NKP, 2, 128)                 # hd, kp, j, p
        return np.ascontiguousarray(arr.transpose(1, 3, 2, 0)).reshape(NKP, 128, 2048)
    return pk(w8), pk(wlo)


def marshal(inputs):
    x = np.asarray(inputs["x"], np.float32)
    wq = np.asarray(inputs["wq"], np.float32)
    wkv = np.asarray(inputs["wkv"], np.float32)
    wo = np.asarray(inputs["wo"], np.float32)
    bq = np.asarray(inputs["bq"], np.float32)
    bkv = np.asarray(inputs["bkv"], np.float32)
    alibi = np.asarray(inputs["alibi_slopes"], np.float32)
    wk_full, wv_full = wkv[:C], wkv[C:]
    bk_full, bv_full = bkv[:C], bkv[C:]

    perm = np.concatenate([np.arange(0, D, 2), np.arange(1, D, 2)])
    head_perm = np.concatenate([h * D + perm for h in range(H)])
    wq_p, wk_p = wq[head_perm], wk_full[head_perm]
    bq_p, bk_p = bq[head_perm], bk_full[head_perm]

    t_abs = np.arange(W, W + L, dtype=np.float64)
    inv = 1.0 / (10000.0 ** (np.arange(0, D, 2, dtype=np.float64) / D))
    fr = np.outer(t_abs, inv)
    cosT = np.cos(fr).T
    sinT = np.sin(fr).T
    cos2 = np.concatenate([cosT, cosT], 0).astype(BF16NP)
    # rows 0:64 feed t2[64:128] (+sin), rows 64:128 feed t2[0:64] (-sin)
    sinsw = np.concatenate([sinT, -sinT], 0).astype(BF16NP)

    dj = np.arange(128)[:, None]
    y = np.arange(MASK_W)[None, :]
    rel = (dj - y + MASK_C0).astype(np.float64)
    win = (rel <= 0) & (rel >= -W)

    in_maps = []
    for core in range(8):
        b, g = divmod(core, 2)
        gs = slice(g * GD, (g + 1) * GD)
        x8m, xlom = pack_x(x[:, b, :])
        x8m = np.concatenate([x8m, np.full((1, 128, 2048), 1.0 / 16, FP8NP)], 0)
        wq8m, wqlom = pack_wqk(WS * wq_p[gs])
        wk8m, wklom = pack_wqk(WS * wk_p[gs])
        wv8m, wvlom = pack_wv(WS_V * wv_full[gs])
        bvch = np.broadcast_to((WS_V / 16) * bv_full[gs].astype(np.float32),
                               (128, 2, GD)).reshape(128, 2048).astype(FP8NP)
        wv8m = np.concatenate([wv8m, bvch[None]], 0)
        wo64 = WS * np.ascontiguousarray(wo[:, gs].T)      # (GD d, C c)
        wo8 = q8(wo64)
        wolo = q8(wo64 - wo8.astype(np.float32))
        def pk_wo(a):
            return np.ascontiguousarray(
                a.reshape(HPC // 2, 2, 128, C).transpose(0, 2, 1, 3)).reshape(HPC // 2, 128, 2 * C)
        wo8_m, wolo_m = pk_wo(wo8), pk_wo(wolo)
        bq_m = np.ascontiguousarray(WS * bq_p[gs].reshape(HPC, 128).T)
        bk_m = np.ascontiguousarray(WS * bk_p[gs].reshape(HPC, 128).T)
        expbP = np.zeros((HPC, 128, 1536), np.float64)
        for hh in range(HPC):
            s = float(alibi[g * HPC + hh])
            eb = np.where(win, np.exp(s * rel), 0.0)
            for bi, soff in enumerate((256, 512, 768)):
                expbP[hh, :, bi * 512:bi * 512 + 256] = eb[:, soff:soff + 256]
                expbP[hh, :, bi * 512 + 256:(bi + 1) * 512] = eb[:, soff + 128:soff + 384]
        in_maps.append(dict(
            x8=x8m, xlo=xlom, wq8=wq8m, wqlo=wqlom, wk8=wk8m, wklo=wklom,
            wv8=wv8m, wvlo=wvlom, wo8=wo8_m, wolo=wolo_m,
            cos2=cos2, sinsw=sinsw, bq=bq_m, bk=bk_m,
            expbP=expbP.astype(BF16NP),
            ones=np.ones((128, 128), BF16NP)))
    return in_maps


def gather(results, bo):
    bo = np.asarray(bo, np.float32)
    out = np.empty((L, N, C), np.float32)
    for b in range(N):
        out[:, b, :] = results[2 * b]["out"] + results[2 * b + 1]["out"] + bo[None, :]
    return out


_NC_CACHE = {}


def _get_nc():
    if "nc" not in _NC_CACHE:
        _NC_CACHE["nc"] = build_nc()
    return _NC_CACHE["nc"]


def kernel(**inputs):
    from concourse import bass_utils
    nc = _get_nc()
    in_maps = marshal(inputs)
    res = bass_utils.run_bass_kernel_spmd(nc, in_maps, core_ids=list(range(8)))
    return gather(res.results, inputs["bo"])


# revision 6
# speedup vs baseline: 1.0105x; 1.0105x over previous
"""Trainium2 Bass kernel: sliding-window causal MHA with RoPE + ALiBi.

Sharding: 8 cores = 4 batches x 2 head-groups (8 heads each).
All large matmuls run as fp8e4m3 DoubleRow (K=256/instruction) with 3-term
residual compensation (hi@hi + lo@hi + hi@lo), which matches bf16 accuracy at
half the PE row count; attention itself (scores/PV/sums) stays bf16.

Per-core program:
  A1: q/k projections, weights pre-scaled x64 (fp8 range), 1/4096 folded into
      the exp scale. Terms ordered x8-first so PE paces the x8/xlo DMA streams.
      RoPE on bf16 in half-split d-layout (4 DVE ops per head tile).
  A2: v projection (weights x32 so 32*attn stays under fp8 max 240), with an
      extra ones x bias contraction chunk folding bv into v. Interleaved with
      group 0 of attention to keep PE fed.
  C:  per (group, head): paired j-tiles -> one [128,512] scores psum -> exp
      (bf16) -> x expbP (separable ALiBi+window mask blocks, bf16) -> PV +
      ones-matmul sums into one shared psum bank -> reciprocal/mul normalize,
      emitted as fp8 hi (Act) + fp8 residual (DVE) pairs.
  D:  out projection in fp8 DoubleRow (3-term, wo pre-scaled x64), lagged one
      query group behind attention and spread across its head loop; host sums
      the 2 head-group partials + bo.
"""
import sys
sys.path.insert(0, '/opt/trn_rl_repo')

import numpy as np
import ml_dtypes
import concourse.bass as bass
import concourse.bacc as bacc
import concourse.mybir as mybir
import concourse.tile as tile

L, N, C, H, D, W = 1024, 4, 2048, 16, 128, 512
HPC = 8                       # heads per core
GD = HPC * D                  # 1024 head-dims per core
NKP = C // 256                # 8 contraction k-pairs (256 each)
SCALE = 1.0 / float(np.sqrt(D))
WS = 64.0                     # q/k/wo weight pre-scale for fp8 range
WS_V = 32.0                   # v-path pre-scale (keeps 32*attn under fp8e4m3 max 240)
F32 = mybir.dt.float32
BF16 = mybir.dt.bfloat16
FP8 = mybir.dt.float8e4
DRM = mybir.MatmulPerfMode.DoubleRow
AF = mybir.ActivationFunctionType
ALU = mybir.AluOpType
QG = 256
NQG = L // QG                 # 4
MASK_W = 1408
MASK_C0 = 384


def jtiles(i0):
    return list(range(max(0, i0 - W), min(i0 + QG, L) - 128 + 1, 128))


def pair2(i):
    return i.rearrange("p (two n) -> p two n", two=2)


def emit(tc, t):
    nc = tc.nc
    cpool = tc.alloc_tile_pool(name="const", bufs=1, side="left")
    bq_s = cpool.tile([128, HPC], F32, tag="bq")
    bk_s = cpool.tile([128, HPC], F32, tag="bk")
    ones = cpool.tile([128, 128], BF16, tag="ones")
    expbs = [cpool.tile([128, 1536], BF16, tag=f"eb{h}", name=f"eb{h}")
             for h in range(HPC)]

    # persistent left-stack results
    qkp = tc.alloc_tile_pool(name="qkp", bufs=1, side="left")
    qts = [qkp.tile([128, L], BF16, tag=f"q{m}", name=f"q{m}") for m in range(HPC)]
    kts = [qkp.tile([128, L], BF16, tag=f"k{m}", name=f"k{m}") for m in range(HPC)]
    vp = tc.alloc_tile_pool(name="vp", bufs=1, side="left")
    vts = [vp.tile([128, GD], BF16, tag=f"v{tt}", name=f"v{tt}") for tt in range(HPC)]
    # attention outputs as fp8 hi/lo pairs, head-pair-major for DR out-proj:
    # at8p[hp][p, (h%2)*1024 + t]
    atp = tc.alloc_tile_pool(name="atp", bufs=1, side="left")
    at8p = [atp.tile([128, 2048], FP8, tag=f"a8{hp}", name=f"a8{hp}")
            for hp in range(HPC // 2)]
    atlop = [atp.tile([128, 2048], FP8, tag=f"alo{hp}", name=f"alo{hp}")
             for hp in range(HPC // 2)]

    # fp8 inputs (right stack, released after v-proj); chunk NKP of x8/wv8 is
    # the ones/bias pair that folds bv into the v projection
    xp = tc.alloc_tile_pool(name="xp", bufs=1, side="right")
    x8s = xp.tile([128, (NKP + 1) * 2048], FP8, tag="x8")
    xlos = xp.tile([128, NKP * 2048], FP8, tag="xlo")
    wvp = tc.alloc_tile_pool(name="wvp", bufs=1, side="right")
    wv8s = wvp.tile([128, (NKP + 1) * 2048], FP8, tag="wv8")
    wvlos = wvp.tile([128, NKP * 2048], FP8, tag="wvlo")

    # ---------------- phase A1: q/k projections + rope ----------------
    # DMA order: x8 chunks, job0 weights, xlo chunks, constants, then
    # weights prefetched one (m, proj) job ahead. Within a job the chains
    # run term-major so the two x8-only terms overlap the xlo stream.
    with tc.tile_pool(name="csp", bufs=1, side="right") as csp, \
         tc.tile_pool(name="ws", bufs=6, side="right") as ws, \
         tc.tile_pool(name="rp", bufs=3, side="right") as rp, \
         tc.tile_pool(name="pa", bufs=6, space="PSUM") as pa:
        cos2 = csp.tile([128, L], BF16, tag="cos2")
        sinsw = csp.tile([128, L], BF16, tag="sinsw")
        def fetch_w(pair_m, interleave_x0=False):
            out = []
            for wname, wloname in (("wq8", "wqlo"), ("wk8", "wklo")):
                w8t = ws.tile([128, 2048], FP8, tag="w8", name="w8")
                nc.sync.dma_start(w8t[:], t[wname][pair_m])
                if interleave_x0 and wname == "wq8":
                    nc.sync.dma_start(x8s[:, 0:2048], t["x8"][0])
                wlot = ws.tile([128, 2048], FP8, tag="wlo", name="wlo")
                nc.sync.dma_start(wlot[:], t[wloname][pair_m])
                out.append((w8t, wlot))
            return out

        wq_fifo = [fetch_w(0, interleave_x0=True)]
        for kp in range(1, NKP + 1):
            nc.sync.dma_start(x8s[:, kp * 2048:(kp + 1) * 2048], t["x8"][kp])
        for kp in range(NKP):
            nc.sync.dma_start(xlos[:, kp * 2048:(kp + 1) * 2048], t["xlo"][kp])
        wq_fifo.append(fetch_w(1))
        nc.sync.dma_start(cos2[:], t["cos2"][:])
        nc.sync.dma_start(sinsw[:], t["sinsw"][:])
        nc.sync.dma_start(bq_s[:], t["bq"][:])
        nc.sync.dma_start(bk_s[:], t["bk"][:])
        nc.sync.dma_start(ones[:], t["ones"][:])

        for m in range(HPC):
            wpair = wq_fifo.pop(0)
            if m + 2 < HPC:
                wq_fifo.append(fetch_w(m + 2))
            # q and k jobs of this m: 4 open psums; the two x8-only terms of
            # both jobs run first so PE keeps pace with the x8/xlo streams.
            pss = [[pa.tile([128, 512], F32, tag="pp", name=f"ps{j}{hf}")
                    for hf in range(2)] for j in range(2)]
            nmm = [[0, 0], [0, 0]]

            def term(j, lhs, rhs, lastterm=False):
                for kp in range(NKP):
                    rh = pair2(rhs[:, kp * 2048:(kp + 1) * 2048])
                    lh = pair2(lhs[:, kp * 256:(kp + 1) * 256])
                    for hf in range(2):
                        for c2 in range(2):
                            tq = hf * 512 + c2 * 256
                            nc.tensor.matmul(
                                pss[j][hf][:, c2 * 256:(c2 + 1) * 256],
                                lh, rh[:, :, tq:tq + 256],
                                start=(nmm[j][hf] == 0),
                                stop=(lastterm and kp == NKP - 1 and c2 == 1),
                                perf_mode=DRM, skip_group_check=True)
                            nmm[j][hf] += 1

            term(0, wpair[0][0], x8s)         # q: w8 @ x8
            term(1, wpair[1][0], x8s)         # k: w8 @ x8
            term(0, wpair[0][1], x8s)         # q: wlo @ x8
            term(1, wpair[1][1], x8s)         # k: wlo @ x8
            term(0, wpair[0][0], xlos, True)  # q: w8 @ xlo
            term(1, wpair[1][0], xlos, True)  # k: w8 @ xlo

            for j, (bias_s, dst) in enumerate(((bq_s, qts), (bk_s, kts))):
                qw = rp.tile([128, L], BF16, tag="qw", name="qw")
                for hf in range(2):
                    nc.scalar.activation(
                        qw[:, hf * 512:(hf + 1) * 512], pss[j][hf][:],
                        AF.Identity, bias=bias_s[:, m:m + 1], scale=1.0)
                # rope on 64-scaled bf16: dst = qw*cos2 + swap(qw)*sin
                t1 = rp.tile([128, L], BF16, tag="t1", name="t1")
                nc.vector.tensor_mul(t1[:], qw[:], cos2[:])
                t2 = rp.tile([128, L], BF16, tag="t2", name="t2")
                nc.vector.tensor_mul(t2[0:64, :], qw[64:128, :], sinsw[64:128, :])
                nc.vector.tensor_mul(t2[64:128, :], qw[0:64, :], sinsw[0:64, :])
                nc.vector.tensor_add(dst[m][:], t1[:], t2[:])

        # wv weights stream in during A1 tail; expb masks for phase C
        for kp in range(NKP + 1):
            nc.sync.dma_start(wv8s[:, kp * 2048:(kp + 1) * 2048], t["wv8"][kp])
        for kp in range(NKP):
            nc.sync.dma_start(wvlos[:, kp * 2048:(kp + 1) * 2048], t["wvlo"][kp])
        for h in range(HPC):
            nc.sync.dma_start(expbs[h][:], t["expbP"][h])

    # out-proj weights on the left stack; DMAs issue right after A1
    wop = tc.alloc_tile_pool(name="wop", bufs=1, side="left")
    wo8t, wolot = [], []
    for hp in range(HPC // 2):
        w8 = wop.tile([128, 4096], FP8, tag="wo8{}".format(hp), name="wo8{}".format(hp))
        nc.sync.dma_start(w8[:], t["wo8"][hp])
        wo8t.append(w8)
    for hp in range(HPC // 2):
        wl = wop.tile([128, 4096], FP8, tag="wolo{}".format(hp), name="wolo{}".format(hp))
        nc.sync.dma_start(wl[:], t["wolo"][hp])
        wolot.append(wl)

    # ------- phases A2 + C + D: v-proj interleaved with group 0, then
    # attention with the out projection lagged one group behind -------
    with tc.tile_pool(name="cwA", bufs=4, side="right") as cwA, \
         tc.tile_pool(name="cwB", bufs=2, side="right") as cwB, \
         tc.tile_pool(name="og", bufs=3, side="right") as og, \
         tc.tile_pool(name="pcs", bufs=4, space="PSUM") as pcs, \
         tc.tile_pool(name="pca", bufs=2, space="PSUM") as pca:

        def attn_head(gi, h):
            i0 = gi * QG
            js = jtiles(i0)
            prs = [(js[2 * u], js[2 * u + 1]) for u in range(len(js) // 2)]
            # one bank: cols [0:256] = PV accum, cols [256:512] = sums accum
            asum = pca.tile([128, 512], F32, tag="asum", name="asum_ps")
            for u, (jlo, jhi) in enumerate(prs):
                bi = (MASK_C0 - (jhi - i0)) // 256 - 1   # 256,512,768 -> 0,1,2
                sp = pcs.tile([128, 512], F32, tag="s", name="s_ps")
                nc.tensor.matmul(sp[:, 0:256], kts[h][:, jhi:jhi + 128],
                                 qts[h][:, i0:i0 + QG], start=True, stop=False,
                                 skip_group_check=True)
                nc.tensor.matmul(sp[:, 256:512], kts[h][:, jlo:jlo + 128],
                                 qts[h][:, i0:i0 + QG], start=False, stop=True,
                                 skip_group_check=True)
                e = cwA.tile([128, 512], BF16, tag="e", name="e")
                nc.scalar.activation(e[:], sp[:], AF.Exp, scale=SCALE / (WS * WS))
                pT = cwA.tile([128, 512], BF16, tag="pT", name="pT")
                nc.vector.tensor_mul(pT[:], e[:], expbs[h][:, bi * 512:(bi + 1) * 512])
                last = (u == len(prs) - 1)
                nc.tensor.matmul(asum[:, 0:256], vts[jhi // 128][:, h * 128:(h + 1) * 128],
                                 pT[:, 0:256], start=(u == 0), stop=False,
                                 skip_group_check=True)
                nc.tensor.matmul(asum[:, 0:256], vts[jlo // 128][:, h * 128:(h + 1) * 128],
                                 pT[:, 256:512], start=False, stop=False,
                                 skip_group_check=True)
                nc.tensor.matmul(asum[:, 256:512], ones[:], pT[:, 0:256],
                                 start=False, stop=False, skip_group_check=True)
                nc.tensor.matmul(asum[:, 256:512], ones[:], pT[:, 256:512],
                                 start=False, stop=last, skip_group_check=True)
            rec = cwB.tile([128, QG], F32, tag="rec", name="rec")
            nc.vector.reciprocal(rec[:], asum[:, 256:512])
            awf = cwB.tile([128, QG], F32, tag="awf", name="awf")
            nc.vector.tensor_mul(awf[:], asum[:, 0:256], rec[:])
            a8sl = at8p[h // 2][:, (h % 2) * 1024 + i0:(h % 2) * 1024 + i0 + QG]
            nc.scalar.activation(a8sl, awf[:], AF.Identity, scale=1.0)
            nc.vector.tensor_sub(
                atlop[h // 2][:, (h % 2) * 1024 + i0:(h % 2) * 1024 + i0 + QG],
                awf[:], a8sl)

        with tc.tile_pool(name="pv", bufs=2, space="PSUM") as pv:
            def v_half(tt, hf):
                ps = pv.tile([128, 512], F32, tag="pp", name="psV")
                nmm2 = 0
                nlast = 2 * (3 * NKP + 1) - 1
                for c2 in range(2):
                    hd0 = hf * 512 + c2 * 256
                    for lhs, rhs, nk in ((x8s, wv8s, NKP + 1), (xlos, wv8s, NKP),
                                         (x8s, wvlos, NKP)):
                        for kp in range(nk):
                            nc.tensor.matmul(
                                ps[:, c2 * 256:(c2 + 1) * 256],
                                pair2(lhs[:, kp * 2048:(kp + 1) * 2048])[:, :, tt * 128:(tt + 1) * 128],
                                pair2(rhs[:, kp * 2048:(kp + 1) * 2048])[:, :, hd0:hd0 + 256],
                                start=(nmm2 == 0), stop=(nmm2 == nlast),
                                perf_mode=DRM, skip_group_check=True)
                            nmm2 += 1
                nc.vector.tensor_copy(vts[tt][:, hf * 512:(hf + 1) * 512], ps[:])

            for tt in (0, 1):
                for hf in range(2):
                    v_half(tt, hf)
            # remaining 12 v half-tiles spread over all 8 gi0 head steps
            sched = [2, 2, 2, 2, 1, 1, 1, 1]
            nxt = 4   # half-tile index (tt = nxt // 2, hf = nxt % 2)
            for h in range(HPC):
                for _ in range(sched[h]):
                    v_half(nxt // 2, nxt % 2)
                    nxt += 1
                attn_head(0, h)

        with tc.tile_pool(name="pd", bufs=2, space="PSUM") as pd:
            def emit_d(gi, idx, pool=None, ptag="po"):
                tt = 2 * gi + idx // 4
                cc = idx % 4
                # fp8 DR out-proj: psum [128,512] = two 256-col chains;
                # each chain: 3 terms x 4 head-pairs, K=256/instr
                ps = (pool or pd).tile([128, 512], F32, tag=ptag, name="psD")
                nmm3 = 0
                for c2 in range(2):
                    c0 = cc * 512 + c2 * 256
                    for lhsl, rhsl in ((at8p, wo8t), (at8p, wolot), (atlop, wo8t)):
                        for hp in range(HPC // 2):
                            nc.tensor.matmul(
                                ps[:, c2 * 256:(c2 + 1) * 256],
                                pair2(lhsl[hp][:, :])[:, :, tt * 128:(tt + 1) * 128],
                                pair2(rhsl[hp][:, :])[:, :, c0:c0 + 256],
                                start=(nmm3 == 0), stop=(nmm3 == 23),
                                perf_mode=DRM, skip_group_check=True)
                            nmm3 += 1
                split = (gi == NQG - 1 and idx == HPC - 1)
                o = og.tile([128, 512], F32, tag="o", name="o")
                if split:
                    # separate evac+DMA per 256-half to shorten the tail
                    for c2 in range(2):
                        sl = slice(c2 * 256, (c2 + 1) * 256)
                        nc.scalar.activation(o[:, sl], ps[:, sl], AF.Identity,
                                             scale=1.0 / (WS_V * WS))
                        nc.sync.dma_start(
                            t["out"][tt * 128:(tt + 1) * 128,
                                     cc * 512 + c2 * 256:cc * 512 + (c2 + 1) * 256],
                            o[:, sl])
                elif gi == NQG - 1:
                    # Act is idle after the last exp; keep DVE free to drain
                    nc.scalar.activation(o[:], ps[:], AF.Identity,
                                         scale=1.0 / (WS_V * WS))
                    nc.sync.dma_start(
                        t["out"][tt * 128:(tt + 1) * 128, cc * 512:(cc + 1) * 512], o[:])
                else:
                    nc.vector.tensor_scalar_mul(o[:], ps[:], 1.0 / (WS_V * WS))
                    nc.sync.dma_start(
                        t["out"][tt * 128:(tt + 1) * 128, cc * 512:(cc + 1) * 512], o[:])

            for gi in range(1, NQG):
                for h in range(HPC):
                    attn_head(gi, h)
                    emit_d(gi - 1, h)
            # attention pools are idle now; alternate psum banks with pcs
            for idx in range(HPC):
                if idx % 2 == 0:
                    emit_d(NQG - 1, idx)
                else:
                    emit_d(NQG - 1, idx, pool=pcs, ptag="s")

    wvp.release()
    xp.release()
    wop.release()
    atp.release()
    vp.release()
    qkp.release()
    cpool.release()


def build_nc(reps=1):
    nc = bacc.Bacc("TRN2", target_bir_lowering=False, debug=False,
                   enable_asserts=False, num_devices=8)
    t = {}
    for name in ("x8", "xlo", "wq8", "wqlo", "wk8", "wklo", "wv8", "wvlo"):
        n0 = 9 if name in ("x8", "wv8") else 8
        t[name] = nc.dram_tensor(name, [n0, 128, 2048], FP8, kind="ExternalInput").ap()
    t["wo8"] = nc.dram_tensor("wo8", [HPC // 2, 128, 4096], FP8, kind="ExternalInput").ap()
    t["wolo"] = nc.dram_tensor("wolo", [HPC // 2, 128, 4096], FP8, kind="ExternalInput").ap()
    t["cos2"] = nc.dram_tensor("cos2", [128, L], BF16, kind="ExternalInput").ap()
    t["sinsw"] = nc.dram_tensor("sinsw", [128, L], BF16, kind="ExternalInput").ap()
    t["bq"] = nc.dram_tensor("bq", [128, HPC], F32, kind="ExternalInput").ap()
    t["bk"] = nc.dram_tensor("bk", [128, HPC], F32, kind="ExternalInput").ap()
    t["expbP"] = nc.dram_tensor("expbP", [HPC, 128, 1536], BF16, kind="ExternalInput").ap()
    t["ones"] = nc.dram_tensor("ones", [128, 128], BF16, kind="ExternalInput").ap()
    t["out"] = nc.dram_tensor("out", [L, C], F32, kind="ExternalOutput").ap()
    with tile.TileContext(nc) as tc:
        for _ in range(reps):
            emit(tc, t)
    nc.compile()
    return nc


FP8NP = ml_dtypes.float8_e4m3
BF16NP = ml_dtypes.bfloat16


def q8(a):
    return a.astype(FP8NP)


def pack_x(xb):
    """xb: (L, C) f32 -> hi/lo fp8 [8, 128, 2048]; [kp][p, j*1024+t]."""
    xT = np.ascontiguousarray(xb.T)            # (C, L)
    x8 = q8(xT)
    xlo = q8(xT - x8.astype(np.float32))
    def pk(a):
        return np.ascontiguousarray(
            a.reshape(NKP, 2, 128, L).transpose(0, 2, 1, 3)).reshape(NKP, 128, 2 * L)
    return pk(x8), pk(xlo)


def pack_wqk(wg):
    """wg: (GD, C) f32 (x64-scaled rows) -> hi/lo fp8 [8 m, 128, 2048];
    [m][p, kp*256 + j*128 + h]."""
    w8 = q8(wg)
    wlo = q8(wg - w8.astype(np.float32))
    def pk(a):
        arr = a.reshape(HPC, 128, NKP, 2, 128)           # m, h, kp, j, p
        return np.ascontiguousarray(arr.transpose(0, 4, 2, 3, 1)).reshape(HPC, 128, 2048)
    return pk(w8), pk(wlo)


def pack_wv(wg):
    """wg: (GD, C) f32 (x64-scaled) -> hi/lo fp8 [8 kp, 128, 2048];
    [kp][p, j*1024 + hd]."""
    w8 = q8(wg)
    wlo = q8(wg - w8.astype(np.float32))
    def pk(a):
        arr = a.reshape(GD, NKP, 2, 128)                 # hd, kp, j, p
        return np.ascontiguousarray(arr.transpose(1, 3, 2, 0)).reshape(NKP, 128, 2048)
    return pk(w8), pk(wlo)


def marshal(inputs):
    x = np.asarray(inputs["x"], np.float32)
    wq = np.asarray(inputs["wq"], np.float32)
    wkv = np.asarray(inputs["wkv"], np.float32)
    wo = np.asarray(inputs["wo"], np.float32)
    bq = np.asarray(inputs["bq"], np.float32)
    bkv = np.asarray(inputs["bkv"], np.float32)
    alibi = np.asarray(inputs["alibi_slopes"], np.float32)
    wk_full, wv_full = wkv[:C], wkv[C:]
    bk_full, bv_full = bkv[:C], bkv[C:]

    perm = np.concatenate([np.arange(0, D, 2), np.arange(1, D, 2)])
    head_perm = np.concatenate([h * D + perm for h in range(H)])
    wq_p, wk_p = wq[head_perm], wk_full[head_perm]
    bq_p, bk_p = bq[head_perm], bk_full[head_perm]

    t_abs = np.arange(W, W + L, dtype=np.float64)
    inv = 1.0 / (10000.0 ** (np.arange(0, D, 2, dtype=np.float64) / D))
    fr = np.outer(t_abs, inv)
    cosT = np.cos(fr).T
    sinT = np.sin(fr).T
    cos2 = np.concatenate([cosT, cosT], 0).astype(BF16NP)
    # rows 0:64 feed t2[64:128] (+sin), rows 64:128 feed t2[0:64] (-sin)
    sinsw = np.concatenate([sinT, -sinT], 0).astype(BF16NP)

    dj = np.arange(128)[:, None]
    y = np.arange(MASK_W)[None, :]
    rel = (dj - y + MASK_C0).astype(np.float64)
    win = (rel <= 0) & (rel >= -W)

    in_maps = []
    for core in range(8):
        b, g = divmod(core, 2)
        gs = slice(g * GD, (g + 1) * GD)
        x8m, xlom = pack_x(x[:, b, :])
        x8m = np.concatenate([x8m, np.full((1, 128, 2048), 1.0 / 16, FP8NP)], 0)
        wq8m, wqlom = pack_wqk(WS * wq_p[gs])
        wk8m, wklom = pack_wqk(WS * wk_p[gs])
        wv8m, wvlom = pack_wv(WS_V * wv_full[gs])
        bvch = np.broadcast_to((WS_V / 16) * bv_full[gs].astype(np.float32),
                               (128, 2, GD)).reshape(128, 2048).astype(FP8NP)
        wv8m = np.concatenate([wv8m, bvch[None]], 0)
        wo64 = WS * np.ascontiguousarray(wo[:, gs].T)      # (GD d, C c)
        wo8 = q8(wo64)
        wolo = q8(wo64 - wo8.astype(np.float32))
        def pk_wo(a):
            return np.ascontiguousarray(
                a.reshape(HPC // 2, 2, 128, C).transpose(0, 2, 1, 3)).reshape(HPC // 2, 128, 2 * C)
        wo8_m, wolo_m = pk_wo(wo8), pk_wo(wolo)
        bq_m = np.ascontiguousarray(WS * bq_p[gs].reshape(HPC, 128).T)
        bk_m = np.ascontiguousarray(WS * bk_p[gs].reshape(HPC, 128).T)
        expbP = np.zeros((HPC, 128, 1536), np.float64)
        for hh in range(HPC):
            s = float(alibi[g * HPC + hh])
            eb = np.where(win, np.exp(s * rel), 0.0)
            for bi, soff in enumerate((256, 512, 768)):
                expbP[hh, :, bi * 512:bi * 512 + 256] = eb[:, soff:soff + 256]
                expbP[hh, :, bi * 512 + 256:(bi + 1) * 512] = eb[:, soff + 128:soff + 384]
        in_maps.append(dict(
            x8=x8m, xlo=xlom, wq8=wq8m, wqlo=wqlom, wk8=wk8m, wklo=wklom,
            wv8=wv8m, wvlo=wvlom, wo8=wo8_m, wolo=wolo_m,
            cos2=cos2, sinsw=sinsw, bq=bq_m, bk=bk_m,
            expbP=expbP.astype(BF16NP),
            ones=np.ones((128, 128), BF16NP)))
    return in_maps


def gather(results, bo):
    bo = np.asarray(bo, np.float32)
    out = np.empty((L, N, C), np.float32)
    for b in range(N):
        out[:, b, :] = results[2 * b]["out"] + results[2 * b + 1]["out"] + bo[None, :]
    return out


_NC_CACHE = {}


def _get_nc():
    if "nc" not in _NC_CACHE:
        _NC_CACHE["nc"] = build_nc()
    return _NC_CACHE["nc"]


def kernel(**inputs):
    from concourse import bass_utils
    nc = _get_nc()
    in_maps = marshal(inputs)
    res = bass_utils.run_bass_kernel_spmd(nc, in_maps, core_ids=list(range(8)))
    return gather(res.results, inputs["bo"])


# revision 7
# speedup vs baseline: 1.0109x; 1.0003x over previous
"""Trainium2 Bass kernel: sliding-window causal MHA with RoPE + ALiBi.

Sharding: 8 cores = 4 batches x 2 head-groups (8 heads each).
All large matmuls run as fp8e4m3 DoubleRow (K=256/instruction) with 3-term
residual compensation (hi@hi + lo@hi + hi@lo), which matches bf16 accuracy at
half the PE row count; attention itself (scores/PV/sums) stays bf16.

Per-core program:
  A1: q/k projections, weights pre-scaled x64 (fp8 range), 1/4096 folded into
      the exp scale. Terms ordered x8-first so PE paces the x8/xlo DMA streams.
      RoPE on bf16 in half-split d-layout (4 DVE ops per head tile).
  A2: v projection (weights x32 so 32*attn stays under fp8 max 240), with an
      extra ones x bias contraction chunk folding bv into v. Interleaved with
      group 0 of attention to keep PE fed.
  C:  per (group, head): paired j-tiles -> one [128,512] scores psum -> exp
      (bf16) -> x expbP (separable ALiBi+window mask blocks, bf16) -> PV +
      ones-matmul sums into one shared psum bank -> reciprocal/mul normalize,
      emitted as fp8 hi (Act) + fp8 residual (DVE) pairs.
  D:  out projection in fp8 DoubleRow (3-term, wo pre-scaled x64), lagged one
      query group behind attention and spread across its head loop; host sums
      the 2 head-group partials + bo.
"""
import sys
sys.path.insert(0, '/opt/trn_rl_repo')

import numpy as np
import ml_dtypes
import concourse.bass as bass
import concourse.bacc as bacc
import concourse.mybir as mybir
import concourse.tile as tile

L, N, C, H, D, W = 1024, 4, 2048, 16, 128, 512
HPC = 8                       # heads per core
GD = HPC * D                  # 1024 head-dims per core
NKP = C // 256                # 8 contraction k-pairs (256 each)
SCALE = 1.0 / float(np.sqrt(D))
WS = 64.0                     # q/k/wo weight pre-scale for fp8 range
WS_V = 32.0                   # v-path pre-scale (keeps 32*attn under fp8e4m3 max 240)
F32 = mybir.dt.float32
BF16 = mybir.dt.bfloat16
FP8 = mybir.dt.float8e4
DRM = mybir.MatmulPerfMode.DoubleRow
AF = mybir.ActivationFunctionType
ALU = mybir.AluOpType
QG = 256
NQG = L // QG                 # 4
MASK_W = 1408
MASK_C0 = 384


def jtiles(i0):
    return list(range(max(0, i0 - W), min(i0 + QG, L) - 128 + 1, 128))


def pair2(i):
    return i.rearrange("p (two n) -> p two n", two=2)


def emit(tc, t, vbias):
    nxc = NKP + (1 if vbias else 0)   # x8/wv8 chunk count
    nc = tc.nc
    cpool = tc.alloc_tile_pool(name="const", bufs=1, side="left")
    bq_s = cpool.tile([128, HPC], F32, tag="bq")
    bk_s = cpool.tile([128, HPC], F32, tag="bk")
    ones = cpool.tile([128, 128], BF16, tag="ones")
    expbs = [cpool.tile([128, 1536], BF16, tag=f"eb{h}", name=f"eb{h}")
             for h in range(HPC)]

    # persistent left-stack results
    qkp = tc.alloc_tile_pool(name="qkp", bufs=1, side="left")
    qts = [qkp.tile([128, L], BF16, tag=f"q{m}", name=f"q{m}") for m in range(HPC)]
    kts = [qkp.tile([128, L], BF16, tag=f"k{m}", name=f"k{m}") for m in range(HPC)]
    vp = tc.alloc_tile_pool(name="vp", bufs=1, side="left")
    vts = [vp.tile([128, GD], BF16, tag=f"v{tt}", name=f"v{tt}") for tt in range(HPC)]
    # attention outputs as fp8 hi/lo pairs, head-pair-major for DR out-proj:
    # at8p[hp][p, (h%2)*1024 + t]
    atp = tc.alloc_tile_pool(name="atp", bufs=1, side="left")
    at8p = [atp.tile([128, 2048], FP8, tag=f"a8{hp}", name=f"a8{hp}")
            for hp in range(HPC // 2)]
    atlop = [atp.tile([128, 2048], FP8, tag=f"alo{hp}", name=f"alo{hp}")
             for hp in range(HPC // 2)]

    # fp8 inputs (right stack, released after v-proj); chunk NKP of x8/wv8 is
    # the ones/bias pair that folds bv into the v projection
    xp = tc.alloc_tile_pool(name="xp", bufs=1, side="right")
    x8s = xp.tile([128, nxc * 2048], FP8, tag="x8")
    xlos = xp.tile([128, NKP * 2048], FP8, tag="xlo")
    wvp = tc.alloc_tile_pool(name="wvp", bufs=1, side="right")
    wv8s = wvp.tile([128, nxc * 2048], FP8, tag="wv8")
    wvlos = wvp.tile([128, NKP * 2048], FP8, tag="wvlo")

    # ---------------- phase A1: q/k projections + rope ----------------
    # DMA order: x8 chunks, job0 weights, xlo chunks, constants, then
    # weights prefetched one (m, proj) job ahead. Within a job the chains
    # run term-major so the two x8-only terms overlap the xlo stream.
    with tc.tile_pool(name="csp", bufs=1, side="right") as csp, \
         tc.tile_pool(name="ws", bufs=6, side="right") as ws, \
         tc.tile_pool(name="rp", bufs=3, side="right") as rp, \
         tc.tile_pool(name="pa", bufs=6, space="PSUM") as pa:
        cos2 = csp.tile([128, L], BF16, tag="cos2")
        sinsw = csp.tile([128, L], BF16, tag="sinsw")
        def fetch_w(pair_m, interleave_x0=False):
            out = []
            for wname, wloname in (("wq8", "wqlo"), ("wk8", "wklo")):
                w8t = ws.tile([128, 2048], FP8, tag="w8", name="w8")
                nc.sync.dma_start(w8t[:], t[wname][pair_m])
                if interleave_x0 and wname == "wq8":
                    nc.sync.dma_start(x8s[:, 0:2048], t["x8"][0])
                wlot = ws.tile([128, 2048], FP8, tag="wlo", name="wlo")
                nc.sync.dma_start(wlot[:], t[wloname][pair_m])
                out.append((w8t, wlot))
            return out

        wq_fifo = [fetch_w(0, interleave_x0=True)]
        for kp in range(1, nxc):
            nc.sync.dma_start(x8s[:, kp * 2048:(kp + 1) * 2048], t["x8"][kp])
        for kp in range(NKP):
            nc.sync.dma_start(xlos[:, kp * 2048:(kp + 1) * 2048], t["xlo"][kp])
        wq_fifo.append(fetch_w(1))
        nc.sync.dma_start(cos2[:], t["cos2"][:])
        nc.sync.dma_start(sinsw[:], t["sinsw"][:])
        nc.sync.dma_start(bq_s[:], t["bq"][:])
        nc.sync.dma_start(bk_s[:], t["bk"][:])
        nc.sync.dma_start(ones[:], t["ones"][:])

        for m in range(HPC):
            wpair = wq_fifo.pop(0)
            if m + 2 < HPC:
                wq_fifo.append(fetch_w(m + 2))
            # q and k jobs of this m: 4 open psums; the two x8-only terms of
            # both jobs run first so PE keeps pace with the x8/xlo streams.
            pss = [[pa.tile([128, 512], F32, tag="pp", name=f"ps{j}{hf}")
                    for hf in range(2)] for j in range(2)]
            nmm = [[0, 0], [0, 0]]

            def term(j, lhs, rhs, lastterm=False):
                for kp in range(NKP):
                    rh = pair2(rhs[:, kp * 2048:(kp + 1) * 2048])
                    lh = pair2(lhs[:, kp * 256:(kp + 1) * 256])
                    for hf in range(2):
                        for c2 in range(2):
                            tq = hf * 512 + c2 * 256
                            nc.tensor.matmul(
                                pss[j][hf][:, c2 * 256:(c2 + 1) * 256],
                                lh, rh[:, :, tq:tq + 256],
                                start=(nmm[j][hf] == 0),
                                stop=(lastterm and kp == NKP - 1 and c2 == 1),
                                perf_mode=DRM, skip_group_check=True)
                            nmm[j][hf] += 1

            term(0, wpair[0][0], x8s)         # q: w8 @ x8
            term(1, wpair[1][0], x8s)         # k: w8 @ x8
            term(0, wpair[0][1], x8s)         # q: wlo @ x8
            term(1, wpair[1][1], x8s)         # k: wlo @ x8
            term(0, wpair[0][0], xlos, True)  # q: w8 @ xlo
            term(1, wpair[1][0], xlos, True)  # k: w8 @ xlo

            for j, (bias_s, dst) in enumerate(((bq_s, qts), (bk_s, kts))):
                qw = rp.tile([128, L], BF16, tag="qw", name="qw")
                for hf in range(2):
                    nc.scalar.activation(
                        qw[:, hf * 512:(hf + 1) * 512], pss[j][hf][:],
                        AF.Identity, bias=bias_s[:, m:m + 1], scale=1.0)
                # rope on 64-scaled bf16: dst = qw*cos2 + swap(qw)*sin
                t1 = rp.tile([128, L], BF16, tag="t1", name="t1")
                nc.vector.tensor_mul(t1[:], qw[:], cos2[:])
                t2 = rp.tile([128, L], BF16, tag="t2", name="t2")
                nc.vector.tensor_mul(t2[0:64, :], qw[64:128, :], sinsw[64:128, :])
                nc.vector.tensor_mul(t2[64:128, :], qw[0:64, :], sinsw[0:64, :])
                nc.vector.tensor_add(dst[m][:], t1[:], t2[:])

        # wv weights stream in during A1 tail; expb masks for phase C
        for kp in range(nxc):
            nc.sync.dma_start(wv8s[:, kp * 2048:(kp + 1) * 2048], t["wv8"][kp])
        for kp in range(NKP):
            nc.sync.dma_start(wvlos[:, kp * 2048:(kp + 1) * 2048], t["wvlo"][kp])
        for h in range(HPC):
            nc.sync.dma_start(expbs[h][:], t["expbP"][h])

    # out-proj weights on the left stack; DMAs issue right after A1
    wop = tc.alloc_tile_pool(name="wop", bufs=1, side="left")
    wo8t, wolot = [], []
    for hp in range(HPC // 2):
        w8 = wop.tile([128, 4096], FP8, tag="wo8{}".format(hp), name="wo8{}".format(hp))
        nc.sync.dma_start(w8[:], t["wo8"][hp])
        wo8t.append(w8)
    for hp in range(HPC // 2):
        wl = wop.tile([128, 4096], FP8, tag="wolo{}".format(hp), name="wolo{}".format(hp))
        nc.sync.dma_start(wl[:], t["wolo"][hp])
        wolot.append(wl)

    # ------- phases A2 + C + D: v-proj interleaved with group 0, then
    # attention with the out projection lagged one group behind -------
    with tc.tile_pool(name="cwA", bufs=4, side="right") as cwA, \
         tc.tile_pool(name="cwB", bufs=2, side="right") as cwB, \
         tc.tile_pool(name="og", bufs=3, side="right") as og, \
         tc.tile_pool(name="pcs", bufs=4, space="PSUM") as pcs, \
         tc.tile_pool(name="pca", bufs=2, space="PSUM") as pca:

        def attn_head(gi, h):
            i0 = gi * QG
            js = jtiles(i0)
            prs = [(js[2 * u], js[2 * u + 1]) for u in range(len(js) // 2)]
            # one bank: cols [0:256] = PV accum, cols [256:512] = sums accum
            asum = pca.tile([128, 512], F32, tag="asum", name="asum_ps")
            for u, (jlo, jhi) in enumerate(prs):
                bi = (MASK_C0 - (jhi - i0)) // 256 - 1   # 256,512,768 -> 0,1,2
                sp = pcs.tile([128, 512], F32, tag="s", name="s_ps")
                nc.tensor.matmul(sp[:, 0:256], kts[h][:, jhi:jhi + 128],
                                 qts[h][:, i0:i0 + QG], start=True, stop=False,
                                 skip_group_check=True)
                nc.tensor.matmul(sp[:, 256:512], kts[h][:, jlo:jlo + 128],
                                 qts[h][:, i0:i0 + QG], start=False, stop=True,
                                 skip_group_check=True)
                e = cwA.tile([128, 512], BF16, tag="e", name="e")
                nc.scalar.activation(e[:], sp[:], AF.Exp, scale=SCALE / (WS * WS))
                pT = cwA.tile([128, 512], BF16, tag="pT", name="pT")
                nc.vector.tensor_mul(pT[:], e[:], expbs[h][:, bi * 512:(bi + 1) * 512])
                last = (u == len(prs) - 1)
                nc.tensor.matmul(asum[:, 0:256], vts[jhi // 128][:, h * 128:(h + 1) * 128],
                                 pT[:, 0:256], start=(u == 0), stop=False,
                                 skip_group_check=True)
                nc.tensor.matmul(asum[:, 0:256], vts[jlo // 128][:, h * 128:(h + 1) * 128],
                                 pT[:, 256:512], start=False, stop=False,
                                 skip_group_check=True)
                nc.tensor.matmul(asum[:, 256:512], ones[:], pT[:, 0:256],
                                 start=False, stop=False, skip_group_check=True)
                nc.tensor.matmul(asum[:, 256:512], ones[:], pT[:, 256:512],
                                 start=False, stop=last, skip_group_check=True)
            rec = cwB.tile([128, QG], F32, tag="rec", name="rec")
            nc.vector.reciprocal(rec[:], asum[:, 256:512])
            awf = cwB.tile([128, QG], F32, tag="awf", name="awf")
            nc.vector.tensor_mul(awf[:], asum[:, 0:256], rec[:])
            a8sl = at8p[h // 2][:, (h % 2) * 1024 + i0:(h % 2) * 1024 + i0 + QG]
            nc.scalar.activation(a8sl, awf[:], AF.Identity, scale=1.0)
            nc.vector.tensor_sub(
                atlop[h // 2][:, (h % 2) * 1024 + i0:(h % 2) * 1024 + i0 + QG],
                awf[:], a8sl)

        with tc.tile_pool(name="pv", bufs=2, space="PSUM") as pv:
            def v_half(tt, hf):
                ps = pv.tile([128, 512], F32, tag="pp", name="psV")
                nmm2 = 0
                nlast = 2 * (2 * NKP + nxc) - 1
                for c2 in range(2):
                    hd0 = hf * 512 + c2 * 256
                    for lhs, rhs, nk in ((x8s, wv8s, nxc), (xlos, wv8s, NKP),
                                         (x8s, wvlos, NKP)):
                        for kp in range(nk):
                            nc.tensor.matmul(
                                ps[:, c2 * 256:(c2 + 1) * 256],
                                pair2(lhs[:, kp * 2048:(kp + 1) * 2048])[:, :, tt * 128:(tt + 1) * 128],
                                pair2(rhs[:, kp * 2048:(kp + 1) * 2048])[:, :, hd0:hd0 + 256],
                                start=(nmm2 == 0), stop=(nmm2 == nlast),
                                perf_mode=DRM, skip_group_check=True)
                            nmm2 += 1
                nc.vector.tensor_copy(vts[tt][:, hf * 512:(hf + 1) * 512], ps[:])

            for tt in (0, 1):
                for hf in range(2):
                    v_half(tt, hf)
            # remaining 12 v half-tiles spread over all 8 gi0 head steps
            sched = [2, 2, 2, 2, 1, 1, 1, 1]
            nxt = 4   # half-tile index (tt = nxt // 2, hf = nxt % 2)
            for h in range(HPC):
                for _ in range(sched[h]):
                    v_half(nxt // 2, nxt % 2)
                    nxt += 1
                attn_head(0, h)

        with tc.tile_pool(name="pd", bufs=2, space="PSUM") as pd:
            def emit_d(gi, idx, pool=None, ptag="po"):
                tt = 2 * gi + idx // 4
                cc = idx % 4
                # fp8 DR out-proj: psum [128,512] = two 256-col chains;
                # each chain: 3 terms x 4 head-pairs, K=256/instr
                ps = (pool or pd).tile([128, 512], F32, tag=ptag, name="psD")
                nmm3 = 0
                for c2 in range(2):
                    c0 = cc * 512 + c2 * 256
                    for lhsl, rhsl in ((at8p, wo8t), (at8p, wolot), (atlop, wo8t)):
                        for hp in range(HPC // 2):
                            nc.tensor.matmul(
                                ps[:, c2 * 256:(c2 + 1) * 256],
                                pair2(lhsl[hp][:, :])[:, :, tt * 128:(tt + 1) * 128],
                                pair2(rhsl[hp][:, :])[:, :, c0:c0 + 256],
                                start=(nmm3 == 0), stop=(nmm3 == 23),
                                perf_mode=DRM, skip_group_check=True)
                            nmm3 += 1
                split = (gi == NQG - 1 and idx == HPC - 1)
                o = og.tile([128, 512], F32, tag="o", name="o")
                if split:
                    # separate evac+DMA per 256-half to shorten the tail
                    for c2 in range(2):
                        sl = slice(c2 * 256, (c2 + 1) * 256)
                        nc.scalar.activation(o[:, sl], ps[:, sl], AF.Identity,
                                             scale=1.0 / (WS_V * WS))
                        nc.sync.dma_start(
                            t["out"][tt * 128:(tt + 1) * 128,
                                     cc * 512 + c2 * 256:cc * 512 + (c2 + 1) * 256],
                            o[:, sl])
                elif gi == NQG - 1:
                    # Act is idle after the last exp; keep DVE free to drain
                    nc.scalar.activation(o[:], ps[:], AF.Identity,
                                         scale=1.0 / (WS_V * WS))
                    nc.sync.dma_start(
                        t["out"][tt * 128:(tt + 1) * 128, cc * 512:(cc + 1) * 512], o[:])
                else:
                    nc.vector.tensor_scalar_mul(o[:], ps[:], 1.0 / (WS_V * WS))
                    nc.sync.dma_start(
                        t["out"][tt * 128:(tt + 1) * 128, cc * 512:(cc + 1) * 512], o[:])

            for gi in range(1, NQG):
                for h in range(HPC):
                    attn_head(gi, h)
                    emit_d(gi - 1, h)
            # attention pools are idle now; alternate psum banks with pcs
            for idx in range(HPC):
                if idx % 2 == 0:
                    emit_d(NQG - 1, idx)
                else:
                    emit_d(NQG - 1, idx, pool=pcs, ptag="s")

    wvp.release()
    xp.release()
    wop.release()
    atp.release()
    vp.release()
    qkp.release()
    cpool.release()


def build_nc(reps=1, vbias=False):
    nc = bacc.Bacc("TRN2", target_bir_lowering=False, debug=False,
                   enable_asserts=False, num_devices=8)
    nxc = NKP + (1 if vbias else 0)
    t = {}
    for name in ("x8", "xlo", "wq8", "wqlo", "wk8", "wklo", "wv8", "wvlo"):
        n0 = nxc if name in ("x8", "wv8") else 8
        t[name] = nc.dram_tensor(name, [n0, 128, 2048], FP8, kind="ExternalInput").ap()
    t["wo8"] = nc.dram_tensor("wo8", [HPC // 2, 128, 4096], FP8, kind="ExternalInput").ap()
    t["wolo"] = nc.dram_tensor("wolo", [HPC // 2, 128, 4096], FP8, kind="ExternalInput").ap()
    t["cos2"] = nc.dram_tensor("cos2", [128, L], BF16, kind="ExternalInput").ap()
    t["sinsw"] = nc.dram_tensor("sinsw", [128, L], BF16, kind="ExternalInput").ap()
    t["bq"] = nc.dram_tensor("bq", [128, HPC], F32, kind="ExternalInput").ap()
    t["bk"] = nc.dram_tensor("bk", [128, HPC], F32, kind="ExternalInput").ap()
    t["expbP"] = nc.dram_tensor("expbP", [HPC, 128, 1536], BF16, kind="ExternalInput").ap()
    t["ones"] = nc.dram_tensor("ones", [128, 128], BF16, kind="ExternalInput").ap()
    t["out"] = nc.dram_tensor("out", [L, C], F32, kind="ExternalOutput").ap()
    with tile.TileContext(nc) as tc:
        for _ in range(reps):
            emit(tc, t, vbias)
    nc.compile()
    return nc


FP8NP = ml_dtypes.float8_e4m3
BF16NP = ml_dtypes.bfloat16


def q8(a):
    return a.astype(FP8NP)


def pack_x(xb):
    """xb: (L, C) f32 -> hi/lo fp8 [8, 128, 2048]; [kp][p, j*1024+t]."""
    xT = np.ascontiguousarray(xb.T)            # (C, L)
    x8 = q8(xT)
    xlo = q8(xT - x8.astype(np.float32))
    def pk(a):
        return np.ascontiguousarray(
            a.reshape(NKP, 2, 128, L).transpose(0, 2, 1, 3)).reshape(NKP, 128, 2 * L)
    return pk(x8), pk(xlo)


def pack_wqk(wg):
    """wg: (GD, C) f32 (x64-scaled rows) -> hi/lo fp8 [8 m, 128, 2048];
    [m][p, kp*256 + j*128 + h]."""
    w8 = q8(wg)
    wlo = q8(wg - w8.astype(np.float32))
    def pk(a):
        arr = a.reshape(HPC, 128, NKP, 2, 128)           # m, h, kp, j, p
        return np.ascontiguousarray(arr.transpose(0, 4, 2, 3, 1)).reshape(HPC, 128, 2048)
    return pk(w8), pk(wlo)


def pack_wv(wg):
    """wg: (GD, C) f32 (x64-scaled) -> hi/lo fp8 [8 kp, 128, 2048];
    [kp][p, j*1024 + hd]."""
    w8 = q8(wg)
    wlo = q8(wg - w8.astype(np.float32))
    def pk(a):
        arr = a.reshape(GD, NKP, 2, 128)                 # hd, kp, j, p
        return np.ascontiguousarray(arr.transpose(1, 3, 2, 0)).reshape(NKP, 128, 2048)
    return pk(w8), pk(wlo)


def marshal(inputs, vbias):
    x = np.asarray(inputs["x"], np.float32)
    wq = np.asarray(inputs["wq"], np.float32)
    wkv = np.asarray(inputs["wkv"], np.float32)
    wo = np.asarray(inputs["wo"], np.float32)
    bq = np.asarray(inputs["bq"], np.float32)
    bkv = np.asarray(inputs["bkv"], np.float32)
    alibi = np.asarray(inputs["alibi_slopes"], np.float32)
    wk_full, wv_full = wkv[:C], wkv[C:]
    bk_full, bv_full = bkv[:C], bkv[C:]

    perm = np.concatenate([np.arange(0, D, 2), np.arange(1, D, 2)])
    head_perm = np.concatenate([h * D + perm for h in range(H)])
    wq_p, wk_p = wq[head_perm], wk_full[head_perm]
    bq_p, bk_p = bq[head_perm], bk_full[head_perm]

    t_abs = np.arange(W, W + L, dtype=np.float64)
    inv = 1.0 / (10000.0 ** (np.arange(0, D, 2, dtype=np.float64) / D))
    fr = np.outer(t_abs, inv)
    cosT = np.cos(fr).T
    sinT = np.sin(fr).T
    cos2 = np.concatenate([cosT, cosT], 0).astype(BF16NP)
    # rows 0:64 feed t2[64:128] (+sin), rows 64:128 feed t2[0:64] (-sin)
    sinsw = np.concatenate([sinT, -sinT], 0).astype(BF16NP)

    dj = np.arange(128)[:, None]
    y = np.arange(MASK_W)[None, :]
    rel = (dj - y + MASK_C0).astype(np.float64)
    win = (rel <= 0) & (rel >= -W)

    in_maps = []
    for core in range(8):
        b, g = divmod(core, 2)
        gs = slice(g * GD, (g + 1) * GD)
        x8m, xlom = pack_x(x[:, b, :])
        if vbias:
            x8m = np.concatenate([x8m, np.full((1, 128, 2048), 1.0 / 16, FP8NP)], 0)
        wq8m, wqlom = pack_wqk(WS * wq_p[gs])
        wk8m, wklom = pack_wqk(WS * wk_p[gs])
        wv8m, wvlom = pack_wv(WS_V * wv_full[gs])
        if vbias:
            bvch = np.broadcast_to((WS_V / 16) * bv_full[gs].astype(np.float32),
                                   (128, 2, GD)).reshape(128, 2048).astype(FP8NP)
            wv8m = np.concatenate([wv8m, bvch[None]], 0)
        wo64 = WS * np.ascontiguousarray(wo[:, gs].T)      # (GD d, C c)
        wo8 = q8(wo64)
        wolo = q8(wo64 - wo8.astype(np.float32))
        def pk_wo(a):
            return np.ascontiguousarray(
                a.reshape(HPC // 2, 2, 128, C).transpose(0, 2, 1, 3)).reshape(HPC // 2, 128, 2 * C)
        wo8_m, wolo_m = pk_wo(wo8), pk_wo(wolo)
        bq_m = np.ascontiguousarray(WS * bq_p[gs].reshape(HPC, 128).T)
        bk_m = np.ascontiguousarray(WS * bk_p[gs].reshape(HPC, 128).T)
        expbP = np.zeros((HPC, 128, 1536), np.float64)
        for hh in range(HPC):
            s = float(alibi[g * HPC + hh])
            eb = np.where(win, np.exp(s * rel), 0.0)
            for bi, soff in enumerate((256, 512, 768)):
                expbP[hh, :, bi * 512:bi * 512 + 256] = eb[:, soff:soff + 256]
                expbP[hh, :, bi * 512 + 256:(bi + 1) * 512] = eb[:, soff + 128:soff + 384]
        in_maps.append(dict(
            x8=x8m, xlo=xlom, wq8=wq8m, wqlo=wqlom, wk8=wk8m, wklo=wklom,
            wv8=wv8m, wvlo=wvlom, wo8=wo8_m, wolo=wolo_m,
            cos2=cos2, sinsw=sinsw, bq=bq_m, bk=bk_m,
            expbP=expbP.astype(BF16NP),
            ones=np.ones((128, 128), BF16NP)))
    return in_maps


def gather(results, bo):
    bo = np.asarray(bo, np.float32)
    out = np.empty((L, N, C), np.float32)
    for b in range(N):
        out[:, b, :] = results[2 * b]["out"] + results[2 * b + 1]["out"] + bo[None, :]
    return out


_NC_CACHE = {}


def _get_nc(vbias=False):
    key = ("nc", vbias)
    if key not in _NC_CACHE:
        _NC_CACHE[key] = build_nc(vbias=vbias)
    return _NC_CACHE[key]


def kernel(**inputs):
    from concourse import bass_utils
    # specialize: the ones x bias contraction chunk in the v projection is
    # only emitted when the v bias is actually nonzero
    vbias = bool(np.any(np.asarray(inputs["bkv"], np.float32)[C:]))
    nc = _get_nc(vbias)
    in_maps = marshal(inputs, vbias)
    res = bass_utils.run_bass_kernel_spmd(nc, in_maps, core_ids=list(range(8)))
    return gather(res.results, inputs["bo"])
